# revision 36
# baseline (speedup 1.0000x reference)
"""Trainium2 Bass kernel for nn_AttentionBlock (B=4, N=2048, C=1024, H=16, D=64).

Sharding: 8 cores = (batch b in 0..3) x (head-group g in 0..1), 8 heads per core.
Each core computes the full FiLM-conditioned norm for its batch (redundant
within the pair), QKV projection for its 8 heads, per-head QK-LN + RoPE + SDPA,
and a partial output projection o @ (I + out_w)[rows of its heads].  The host
sums the two partial outputs per batch — no on-device collectives.

Matmuls run in bf16 with fp32 PSUM accumulation; LN statistics, softmax
normalization and all reductions stay fp32.
"""

import numpy as np
import ml_dtypes

import concourse.bass as bass
import concourse.bacc as bacc
import concourse.tile as tile
from concourse import mybir
from concourse.bass import ts
from concourse.bass_utils import run_bass_kernel_spmd
from concourse.masks import make_identity

B, N, C, H, D = 4, 2048, 1024, 16, 64
HG = H // 2            # 8 heads per core
NT = N // 128          # 16 token tiles
KC = C // 128          # 8 contraction chunks over C
EPS = 1e-5
F32 = mybir.dt.float32
BF16 = mybir.dt.bfloat16
AX = mybir.AxisListType.X
OP = mybir.AluOpType
ACTF = mybir.ActivationFunctionType


def build_program(zero_bias: bool):
    nc = bacc.Bacc("TRN2", target_bir_lowering=False)
    x_d = nc.dram_tensor("x", [N, C], F32, kind="ExternalInput")
    embT_d = nc.dram_tensor("embT", [C, N], BF16, kind="ExternalInput")
    embw_d = nc.dram_tensor("emb_w", [C, 2 * C], BF16, kind="ExternalInput")
    projw_d = nc.dram_tensor("proj_w", [C, 3 * 512], BF16, kind="ExternalInput")
    weff_d = nc.dram_tensor("w_eff", [512, C], BF16, kind="ExternalInput")
    cos_d = nc.dram_tensor("cos_t", [N, 32], BF16, kind="ExternalInput")
    sin_d = nc.dram_tensor("sin_t", [N, 32], BF16, kind="ExternalInput")
    if not zero_bias:
        b1p_d = nc.dram_tensor("bias1p", [C], F32, kind="ExternalInput")
        bsh_d = nc.dram_tensor("bias_sh", [C], F32, kind="ExternalInput")
    y_d = nc.dram_tensor("y", [N, C], F32, kind="ExternalOutput")

    x_r = x_d[:, :].rearrange("(t p) c -> p t c", p=128)
    embT_r = embT_d[:, :].rearrange("(ko p) n -> p ko n", p=128)
    embw_r = embw_d[:, :].rearrange("(ko p) n -> p ko n", p=128)
    projw_r = projw_d[:, :].rearrange("(ko p) n -> p ko n", p=128)
    weff_r = weff_d[:, :].rearrange("(ko p) n -> p ko n", p=128)
    cos_r = cos_d[:, :].rearrange("(t p) d -> p t d", p=128)
    sin_r = sin_d[:, :].rearrange("(t p) d -> p t d", p=128)
    y_r = y_d[:, :].rearrange("(t p) c -> p t c", p=128)

    with tile.TileContext(nc) as tc:
        with (
            tc.tile_pool(name="singles", bufs=1) as singles,
            tc.tile_pool(name="persist", bufs=1) as persist,
        ):
            # ---- constants / weights resident in SBUF ----
            emb_w_sb = singles.tile([128, KC, 2 * C], BF16)
            proj_sb = singles.tile([128, KC, 3 * 512], BF16)
            weff_sb = singles.tile([128, 4, C], BF16)
            cos_sb = singles.tile([128, NT, 32], BF16)
            sin_sb = singles.tile([128, NT, 32], BF16)
            ident = singles.tile([128, 128], BF16)
            eps_sb = singles.tile([128, 1], F32)
            one_col = singles.tile([128, 1], F32)

            # chunked weight loads: the first ss matmuls only need the
            # first column chunk, so don't gate them on the full 4MB DMA
            for cc in range(4):
                nc.gpsimd.dma_start(emb_w_sb[:, :, ts(cc, 512)],
                                    embw_r[:, :, ts(cc, 512)])
            for cc in range(3):
                nc.gpsimd.dma_start(proj_sb[:, :, ts(cc, 512)],
                                    projw_r[:, :, ts(cc, 512)])
            nc.gpsimd.dma_start(weff_sb[:], weff_r)
            nc.sync.dma_start(cos_sb[:], cos_r)
            nc.sync.dma_start(sin_sb[:], sin_r)
            make_identity(nc, ident[:])
            nc.vector.memset(eps_sb[:], EPS)
            nc.vector.memset(one_col[:], 1.0)

            if not zero_bias:
                b1p_sb = singles.tile([128, C], F32)
                bsh_sb = singles.tile([128, C], F32)
                b1p_ap = b1p_d[:]
                bsh_ap = bsh_d[:]
                nc.sync.dma_start(
                    b1p_sb[:],
                    bass.AP(tensor=b1p_ap.tensor, offset=b1p_ap.offset,
                            ap=[[0, 128], b1p_ap.ap[0]]),
                )
                nc.sync.dma_start(
                    bsh_sb[:],
                    bass.AP(tensor=bsh_ap.tensor, offset=bsh_ap.offset,
                            ap=[[0, 128], bsh_ap.ap[0]]),
                )

            # ---- per-head persistent activations ----
            # head h lives at partitions 64*(h%2) .. +64, slot h//2
            v_sb = persist.tile([128, HG, NT, 65], BF16)
            qT_sb = persist.tile([128, 4, N], BF16)
            kT_sb = persist.tile([128, 4, N], BF16)
            oTn_sb = persist.tile([128, 4, N], BF16)

            # ================= stage 1: FiLM-norm + QKV + QK-LN/RoPE =========
            with (
                tc.tile_pool(name="work", bufs=3) as work,
                tc.tile_pool(name="qkw", bufs=2) as qkw,
                tc.tile_pool(name="film", bufs=2) as film,
                tc.tile_pool(name="psA", bufs=4, space="PSUM") as psA,
                tc.tile_pool(name="psT", bufs=4, space="PSUM") as psT,
            ):
                def process_qk(ps, which, t):
                    """QK layernorm + RoPE + transpose for 8 heads at once.

                    ps: PSUM [128 tokens, 512 = 8 heads x 64]."""
                    q3 = ps.rearrange("p (h d) -> p h d", h=HG)
                    sumq = qkw.tile([128, HG], F32, tag="sumq")
                    nc.vector.reduce_sum(out=sumq[:], in_=q3, axis=AX)
                    mean = qkw.tile([128, HG], F32, tag="mean")
                    nc.scalar.mul(mean[:], sumq[:], 1.0 / D)
                    # center (PSUM + SBUF -> SBUF), then variance of centered
                    lnq = qkw.tile([128, HG, D], BF16, tag="lnq")
                    mean_b = mean[:, :, None].to_broadcast((128, HG, D))
                    nc.vector.tensor_tensor(lnq[:], q3, mean_b, OP.subtract)
                    qsq = qkw.tile([128, HG, D], BF16, tag="qsq")
                    nc.vector.tensor_tensor(qsq[:], lnq[:], lnq[:], OP.mult)
                    var = qkw.tile([128, HG], F32, tag="var")
                    nc.vector.reduce_sum(out=var[:], in_=qsq[:], axis=AX)
                    nc.scalar.activation(var[:], var[:], ACTF.Sqrt,
                                         bias=eps_sb[:], scale=1.0 / D)
                    nc.vector.reciprocal(var[:], var[:])
                    rstd_b = var[:, :, None].to_broadcast((128, HG, D))
                    nc.vector.tensor_tensor(lnq[:], lnq[:], rstd_b, OP.mult)
                    # RoPE
                    rq = qkw.tile([128, HG, D], BF16, tag="rq")
                    cos_b = cos_sb[:, t, None, :].to_broadcast((128, HG, 32))
                    sin_b = sin_sb[:, t, None, :].to_broadcast((128, HG, 32))
                    ra = qkw.tile([128, HG, 32], BF16, tag="ra")
                    rb = qkw.tile([128, HG, 32], BF16, tag="rb")
                    q1 = lnq[:, :, 0:32]
                    q2 = lnq[:, :, 32:64]
                    nc.vector.tensor_tensor(ra[:], q1, cos_b, OP.mult)
                    nc.vector.tensor_tensor(rb[:], q2, sin_b, OP.mult)
                    nc.vector.tensor_tensor(rq[:, :, 0:32], ra[:], rb[:], OP.subtract)
                    nc.vector.tensor_tensor(ra[:], q1, sin_b, OP.mult)
                    nc.vector.tensor_tensor(rb[:], q2, cos_b, OP.mult)
                    nc.vector.tensor_tensor(rq[:, :, 32:64], ra[:], rb[:], OP.add)
                    # transpose 8 heads; even heads batch into one PSUM tile
                    # (-> partitions 0:64), odd heads into another (-> 64:128)
                    dst = qT_sb if which == 0 else kT_sb
                    ptA = psT.tile([64, 4, 128], BF16, tag="pt", name=f"ptA{which}_{t}")
                    ptB = psT.tile([64, 4, 128], BF16, tag="pt", name=f"ptB{which}_{t}")
                    for h in range(HG):
                        pt = ptA if h % 2 == 0 else ptB
                        nc.tensor.transpose(pt[:, h // 2, :], rq[:, h, :], ident[:])
                    nc.scalar.copy(dst[0:64, :, ts(t, 128)], ptA[:])
                    nc.scalar.copy(dst[64:128, :, ts(t, 128)], ptB[:])

                def part_a(t):
                    """ss matmul + x layernorm + FiLM -> h_bf(t)."""
                    embT_strip = work.tile([128, KC, 128], BF16, tag="embT")
                    nc.sync.dma_start(embT_strip[:], embT_r[:, :, ts(t, 128)])
                    scale1p = film.tile([128, C], BF16, tag="scale")
                    shift = film.tile([128, C], BF16, tag="shift")
                    for cc in range(4):
                        ps = psA.tile([128, 512], F32, tag="mm")
                        for ko in range(KC):
                            nc.tensor.matmul(
                                ps[:], lhsT=embT_strip[:, ko, :],
                                rhs=emb_w_sb[:, ko, ts(cc, 512)],
                                start=(ko == 0), stop=(ko == KC - 1))
                        if zero_bias:
                            if cc < 2:
                                # scale + 1
                                nc.scalar.activation(
                                    scale1p[:, ts(cc, 512)], ps[:],
                                    ACTF.Identity, bias=one_col[:], scale=1.0)
                            else:
                                nc.scalar.copy(shift[:, ts(cc - 2, 512)], ps[:])
                        else:
                            if cc < 2:
                                nc.vector.tensor_tensor(
                                    scale1p[:, ts(cc, 512)], ps[:],
                                    b1p_sb[:, ts(cc, 512)], OP.add)
                            else:
                                nc.vector.tensor_tensor(
                                    shift[:, ts(cc - 2, 512)], ps[:],
                                    bsh_sb[:, ts(cc - 2, 512)], OP.add)
                    # layernorm of x over C
                    x_t = work.tile([128, C], F32, tag="x")
                    nc.sync.dma_start(x_t[:], x_r[:, t, :])
                    stats = work.tile([128, 2, 6], F32, tag="stats")
                    for sg in range(2):
                        nc.vector.bn_stats(stats[:, sg, :], x_t[:, ts(sg, 512)])
                    mv = work.tile([128, 2], F32, tag="mv")
                    nc.vector.bn_aggr(mv[:], stats[:])
                    std = work.tile([128, 1], F32, tag="std")
                    nc.scalar.activation(std[:], mv[:, 1:2], ACTF.Sqrt,
                                         bias=eps_sb[:], scale=1.0)
                    nc.vector.reciprocal(std[:], std[:])
                    # LN (f32 -> bf16), then FiLM fully in bf16 (DVE 2x mode)
                    ln_bf = work.tile([128, C], BF16, tag="lnx")
                    nc.vector.tensor_scalar(
                        out=ln_bf[:], in0=x_t[:], scalar1=mv[:, 0:1], scalar2=std[:],
                        op0=OP.subtract, op1=OP.mult)
                    h_bf = work.tile([128, C], BF16, tag="h")
                    nc.vector.tensor_tensor(ln_bf[:], ln_bf[:], scale1p[:], OP.mult)
                    nc.vector.tensor_tensor(h_bf[:], ln_bf[:], shift[:], OP.add)
                    return h_bf

                def part_b(t, h_bf):
                    """h transpose + QKV + QK-LN/RoPE + v for tile t."""
                    # transpose h for this token tile (batched copies)
                    hT_strip = work.tile([128, KC, 128], BF16, tag="hT")
                    for g4 in range(2):
                        pt = psT.tile([128, 4, 128], BF16, tag="pt",
                                      name=f"pth{g4}_{t}")
                        for j in range(4):
                            nc.tensor.transpose(
                                pt[:, j, :], h_bf[:, ts(g4 * 4 + j, 128)], ident[:])
                        nc.scalar.copy(hT_strip[:, ts(g4, 4), :], pt[:])
                    # QKV projection (q, k, v chunks of 512 cols each)
                    for cc in range(3):
                        ps = psA.tile([128, 512], F32, tag="mm")
                        for kc in range(KC):
                            nc.tensor.matmul(
                                ps[:], lhsT=hT_strip[:, kc, :],
                                rhs=proj_sb[:, kc, ts(cc, 512)],
                                start=(kc == 0), stop=(kc == KC - 1))
                        if cc == 2:
                            nc.scalar.copy(
                                v_sb[:, :, t, 0:64],
                                ps.rearrange("p (h d) -> p h d", h=HG))
                        else:
                            process_qk(ps, cc, t)
                    nc.vector.memset(v_sb[:, :, t, 64:65], 1.0)

                # software pipeline: ss(t+1) on PE overlaps FiLM(t) on DVE
                h_prev = None
                for t in range(NT):
                    h_cur = part_a(t)
                    if h_prev is not None:
                        part_b(t - 1, h_prev)
                    h_prev = h_cur
                part_b(NT - 1, h_prev)

            # ================= stage 2: attention ============================
            with (
                tc.tile_pool(name="attw", bufs=2) as attw,
                tc.tile_pool(name="pexp", bufs=4) as pexp,
                tc.tile_pool(name="psS", bufs=2, space="PSUM") as psS,
                tc.tile_pool(name="psO", bufs=4, space="PSUM") as psO,
            ):
                head_tiles = {}
                rec_d = nc.dram_tensor("rec_scratch", [HG, N], BF16)
                rec_d_ap = rec_d[:, :]

                def emit_norm(h):
                    """Divide head h's o^T rows by its softmax rowsums and DMA
                    to the oTn scratch.  The reciprocal row is broadcast to 64
                    partitions via a DRAM bounce (DMA engines are idle here)."""
                    po = 64 * (h % 2)
                    hs = h // 2
                    oT_u, sums = head_tiles.pop(h)
                    nc.vector.reciprocal(sums[:], sums[:])
                    rec_bf = attw.tile([1, N], BF16, tag="recbf", name=f"recbf{h}")
                    nc.vector.tensor_copy(rec_bf[:], sums[:])
                    nc.sync.dma_start(rec_d_ap[h:h + 1, :], rec_bf[:])
                    recb = attw.tile([64, N], BF16, tag="recb", name=f"recb{h}")
                    src = rec_d_ap[h, :]
                    nc.sync.dma_start(
                        recb[:],
                        bass.AP(tensor=src.tensor, offset=src.offset,
                                ap=[[0, 64], src.ap[0]]))
                    nc.vector.tensor_tensor(
                        oTn_sb[po:po + 64, hs, :], oT_u[:], recb[:], OP.mult)

                for h in range(HG):
                    po = 64 * (h % 2)
                    hs = h // 2
                    oT_ps = [psO.tile([65, 512], F32, tag="oT", name=f"oT{h}_{i}")
                             for i in range(4)]
                    def emit_av(kt, half, pT):
                        for j in range(2):
                            qc = half * 2 + j
                            nc.tensor.matmul(
                                oT_ps[qc][:], lhsT=v_sb[:, h, kt, :],
                                rhs=pT[:, ts(j, 512)],
                                start=(kt == 0), stop=(kt == NT - 1))

                    # av is emitted one step late so exp latency hides under
                    # the next scores matmuls in PE program order
                    pending = None
                    for kt in range(NT):
                        for half in range(2):
                            ps_s = psS.tile([128, 1024], F32, tag="s",
                                            name=f"s{h}_{kt}_{half}")
                            for j in range(2):
                                qc = half * 2 + j
                                nc.tensor.matmul(
                                    ps_s[:, ts(j, 512)],
                                    lhsT=kT_sb[po:po + 64, hs, ts(kt, 128)],
                                    rhs=qT_sb[po:po + 64, hs, ts(qc, 512)],
                                    start=True, stop=True)
                            pT = pexp.tile([128, 1024], BF16, tag="pT",
                                           name=f"pT{h}_{kt}_{half}")
                            nc.scalar.activation(pT[:], ps_s[:], ACTF.Exp,
                                                 scale=0.125)
                            if pending is not None:
                                emit_av(*pending)
                            pending = (kt, half, pT)
                        if kt == 1 and h > 0:
                            emit_norm(h - 1)
                    emit_av(*pending)
                    # copy out accumulated o^T (bf16) and rowsums (f32) on DVE
                    oT_u = attw.tile([64, N], BF16, tag="oTu", name=f"oTu{h}")
                    sums = attw.tile([1, N], F32, tag="sums", name=f"sums{h}")
                    head_tiles[h] = (oT_u, sums)
                    for qc in range(4):
                        nc.vector.tensor_copy(
                            oT_u[:, ts(qc, 512)], oT_ps[qc][0:64, :])
                        nc.vector.tensor_copy(
                            sums[:, ts(qc, 512)], oT_ps[qc][64:65, :])
                emit_norm(HG - 1)

            # ================= stage 3: output projection ====================
            with (
                tc.tile_pool(name="yw", bufs=4) as yw,
                tc.tile_pool(name="psY", bufs=4, space="PSUM") as psY,
            ):
                for t in range(NT):
                    for cc in range(2):
                        ps_y = psY.tile([128, 512], F32, tag="y")
                        for kc in range(4):
                            nc.tensor.matmul(
                                ps_y[:], lhsT=oTn_sb[:, kc, ts(t, 128)],
                                rhs=weff_sb[:, kc, ts(cc, 512)],
                                start=(kc == 0), stop=(kc == 3))
                        y_t = yw.tile([128, 512], F32, tag="yt")
                        nc.scalar.copy(y_t[:], ps_y[:])
                        nc.sync.dma_start(y_r[:, t, ts(cc, 512)], y_t[:])

    nc.compile()
    return nc


_cached = {}


def _get_program(zero_bias):
    if zero_bias not in _cached:
        _cached[zero_bias] = build_program(zero_bias)
    return _cached[zero_bias]


def prepare_in_maps(x, emb, emb_w, emb_b, proj_w, out_w):
    x = np.asarray(x, np.float32)
    emb = np.asarray(emb, np.float32)
    emb_w = np.asarray(emb_w, np.float32)
    emb_b = np.asarray(emb_b, np.float32)
    proj_w = np.asarray(proj_w, np.float32)
    out_w = np.asarray(out_w, np.float32)
    bf = ml_dtypes.bfloat16

    zero_bias = bool(np.all(emb_b == 0.0))
    emb_w_bf = emb_w.astype(bf)
    projs = []
    for g in range(2):
        cols = np.concatenate(
            [proj_w[:, q * C + g * 512: q * C + g * 512 + 512] for q in range(3)],
            axis=1)
        projs.append(np.ascontiguousarray(cols).astype(bf))
    w_eff = np.eye(C, dtype=np.float32) + out_w
    weffs = [np.ascontiguousarray(w_eff[g * 512:(g + 1) * 512, :]).astype(bf)
             for g in range(2)]
    half = D // 2
    inv_freq = (1.0 / (10000.0 ** (np.arange(half, dtype=np.float32) / half)))
    ang = np.arange(N, dtype=np.float32)[:, None] * inv_freq[None, :]
    cos_t = np.cos(ang).astype(bf)
    sin_t = np.sin(ang).astype(bf)
    b1p = (1.0 + emb_b[:C]).astype(np.float32)
    bsh = np.ascontiguousarray(emb_b[C:]).astype(np.float32)

    in_maps = []
    for core in range(8):
        b, g = divmod(core, 2)
        m = {
            "x": np.ascontiguousarray(x[b]),
            "embT": np.ascontiguousarray(emb[b].T).astype(bf),
            "emb_w": emb_w_bf,
            "proj_w": projs[g],
            "w_eff": weffs[g],
            "cos_t": cos_t,
            "sin_t": sin_t,
        }
        if not zero_bias:
            m["bias1p"] = b1p
            m["bias_sh"] = bsh
        in_maps.append(m)
    return in_maps, zero_bias


def kernel(x, emb, emb_w, emb_b, proj_w, out_w, _trace=False):
    in_maps, zero_bias = prepare_in_maps(x, emb, emb_w, emb_b, proj_w, out_w)
    nc = _get_program(zero_bias)
    res = run_bass_kernel_spmd(nc, in_maps, core_ids=list(range(8)),
                               trace=_trace)
    y = np.zeros((B, N, C), np.float32)
    for core in range(8):
        b, g = divmod(core, 2)
        y[b] += res.results[core]["y"]
    if _trace:
        kernel.last_exec_time_ns = res.exec_time_ns
        kernel.last_results = res
    return y


# revision 41
# speedup vs baseline: 1.0365x; 1.0365x over previous
"""Trainium2 Bass kernel for nn_AttentionBlock (B=4, N=2048, C=1024, H=16, D=64).

Sharding: 8 cores = (batch b in 0..3) x (head-group g in 0..1), 8 heads per core.
Each core computes the full FiLM-conditioned norm for its batch (redundant
within the pair), QKV projection for its 8 heads, per-head QK-LN + RoPE + SDPA,
and a partial output projection o @ (I + out_w)[rows of its heads].  The host
sums the two partial outputs per batch — no on-device collectives.

Matmuls run in bf16 with fp32 PSUM accumulation; LN statistics, softmax
normalization and all reductions stay fp32.
"""

import numpy as np
import ml_dtypes

import concourse.bass as bass
import concourse.bacc as bacc
import concourse.tile as tile
from concourse import mybir
from concourse.bass import ts
from concourse.bass_utils import run_bass_kernel_spmd
from concourse.masks import make_identity

B, N, C, H, D = 4, 2048, 1024, 16, 64
HG = H // 2            # 8 heads per core
NT = N // 128          # 16 token tiles
KC = C // 128          # 8 contraction chunks over C
EPS = 1e-5
F32 = mybir.dt.float32
BF16 = mybir.dt.bfloat16
AX = mybir.AxisListType.X
OP = mybir.AluOpType
ACTF = mybir.ActivationFunctionType


def build_program(zero_bias: bool):
    nc = bacc.Bacc("TRN2", target_bir_lowering=False)
    x_d = nc.dram_tensor("x", [N, C], F32, kind="ExternalInput")
    embT_d = nc.dram_tensor("embT", [C, N], BF16, kind="ExternalInput")
    embw_d = nc.dram_tensor("emb_w", [C, 2 * C], BF16, kind="ExternalInput")
    projw_d = nc.dram_tensor("proj_w", [C, 3 * 512], BF16, kind="ExternalInput")
    weff_d = nc.dram_tensor("w_eff", [512, C], BF16, kind="ExternalInput")
    cos_d = nc.dram_tensor("cos_t", [N, 32], BF16, kind="ExternalInput")
    sin_d = nc.dram_tensor("sin_t", [N, 32], BF16, kind="ExternalInput")
    if not zero_bias:
        b1p_d = nc.dram_tensor("bias1p", [C], F32, kind="ExternalInput")
        bsh_d = nc.dram_tensor("bias_sh", [C], F32, kind="ExternalInput")
    y_d = nc.dram_tensor("y", [N, C], F32, kind="ExternalOutput")

    x_r = x_d[:, :].rearrange("(t p) c -> p t c", p=128)
    embT_r = embT_d[:, :].rearrange("(ko p) n -> p ko n", p=128)
    embw_r = embw_d[:, :].rearrange("(ko p) n -> p ko n", p=128)
    projw_r = projw_d[:, :].rearrange("(ko p) n -> p ko n", p=128)
    weff_r = weff_d[:, :].rearrange("(ko p) n -> p ko n", p=128)
    cos_r = cos_d[:, :].rearrange("(t p) d -> p t d", p=128)
    sin_r = sin_d[:, :].rearrange("(t p) d -> p t d", p=128)
    y_r = y_d[:, :].rearrange("(t p) c -> p t c", p=128)

    with tile.TileContext(nc) as tc:
        with (
            tc.tile_pool(name="singles", bufs=1) as singles,
            tc.tile_pool(name="persist", bufs=1) as persist,
        ):
            # ---- constants / weights resident in SBUF ----
            emb_w_sb = singles.tile([128, KC, 2 * C], BF16)
            proj_sb = singles.tile([128, KC, 3 * 512], BF16)
            weff_sb = singles.tile([128, 4, C], BF16)
            cos_sb = singles.tile([128, NT, 32], BF16)
            sin_sb = singles.tile([128, NT, 32], BF16)
            ident = singles.tile([128, 128], BF16)
            eps_sb = singles.tile([128, 1], F32)
            one_col = singles.tile([128, 1], F32)

            # chunked weight loads: the first ss matmuls only need the
            # first column chunk, so don't gate them on the full 4MB DMA
            for cc in range(4):
                nc.gpsimd.dma_start(emb_w_sb[:, :, ts(cc, 512)],
                                    embw_r[:, :, ts(cc, 512)])
            for cc in range(3):
                nc.gpsimd.dma_start(proj_sb[:, :, ts(cc, 512)],
                                    projw_r[:, :, ts(cc, 512)])
            nc.gpsimd.dma_start(weff_sb[:], weff_r)
            nc.sync.dma_start(cos_sb[:], cos_r)
            nc.sync.dma_start(sin_sb[:], sin_r)
            make_identity(nc, ident[:])
            nc.vector.memset(eps_sb[:], EPS)
            nc.vector.memset(one_col[:], 1.0)

            if not zero_bias:
                b1p_sb = singles.tile([128, C], F32)
                bsh_sb = singles.tile([128, C], F32)
                b1p_ap = b1p_d[:]
                bsh_ap = bsh_d[:]
                nc.sync.dma_start(
                    b1p_sb[:],
                    bass.AP(tensor=b1p_ap.tensor, offset=b1p_ap.offset,
                            ap=[[0, 128], b1p_ap.ap[0]]),
                )
                nc.sync.dma_start(
                    bsh_sb[:],
                    bass.AP(tensor=bsh_ap.tensor, offset=bsh_ap.offset,
                            ap=[[0, 128], bsh_ap.ap[0]]),
                )

            # ---- per-head persistent activations ----
            # head h lives at partitions 64*(h%2) .. +64, slot h//2
            v_sb = persist.tile([128, HG, NT, 65], BF16)
            qT_sb = persist.tile([128, 4, N], BF16)
            kT_sb = persist.tile([128, 4, N], BF16)
            oTn_sb = persist.tile([128, 4, N], BF16)

            # ================= stage 1: FiLM-norm + QKV + QK-LN/RoPE =========
            with (
                tc.tile_pool(name="work", bufs=3) as work,
                tc.tile_pool(name="qkw", bufs=2) as qkw,
                tc.tile_pool(name="film", bufs=2) as film,
                tc.tile_pool(name="psA", bufs=4, space="PSUM") as psA,
                tc.tile_pool(name="psT", bufs=4, space="PSUM") as psT,
            ):
                def process_qk(ps, which, t):
                    """QK layernorm + RoPE + transpose for 8 heads at once.

                    ps: PSUM [128 tokens, 512 = 8 heads x 64]."""
                    q3 = ps.rearrange("p (h d) -> p h d", h=HG)
                    sumq = qkw.tile([128, HG], F32, tag="sumq")
                    nc.vector.reduce_sum(out=sumq[:], in_=q3, axis=AX)
                    mean = qkw.tile([128, HG], F32, tag="mean")
                    nc.scalar.mul(mean[:], sumq[:], 1.0 / D)
                    # center (PSUM + SBUF -> SBUF), then variance of centered
                    lnq = qkw.tile([128, HG, D], BF16, tag="lnq")
                    mean_b = mean[:, :, None].to_broadcast((128, HG, D))
                    nc.vector.tensor_tensor(lnq[:], q3, mean_b, OP.subtract)
                    qsq = qkw.tile([128, HG, D], BF16, tag="qsq")
                    nc.vector.tensor_tensor(qsq[:], lnq[:], lnq[:], OP.mult)
                    var = qkw.tile([128, HG], F32, tag="var")
                    nc.vector.reduce_sum(out=var[:], in_=qsq[:], axis=AX)
                    nc.scalar.activation(var[:], var[:], ACTF.Sqrt,
                                         bias=eps_sb[:], scale=1.0 / D)
                    nc.vector.reciprocal(var[:], var[:])
                    rstd_b = var[:, :, None].to_broadcast((128, HG, D))
                    nc.vector.tensor_tensor(lnq[:], lnq[:], rstd_b, OP.mult)
                    # RoPE
                    rq = qkw.tile([128, HG, D], BF16, tag="rq")
                    cos_b = cos_sb[:, t, None, :].to_broadcast((128, HG, 32))
                    sin_b = sin_sb[:, t, None, :].to_broadcast((128, HG, 32))
                    ra = qkw.tile([128, HG, 32], BF16, tag="ra")
                    rb = qkw.tile([128, HG, 32], BF16, tag="rb")
                    q1 = lnq[:, :, 0:32]
                    q2 = lnq[:, :, 32:64]
                    nc.vector.tensor_tensor(ra[:], q1, cos_b, OP.mult)
                    nc.vector.tensor_tensor(rb[:], q2, sin_b, OP.mult)
                    nc.vector.tensor_tensor(rq[:, :, 0:32], ra[:], rb[:], OP.subtract)
                    nc.vector.tensor_tensor(ra[:], q1, sin_b, OP.mult)
                    nc.vector.tensor_tensor(rb[:], q2, cos_b, OP.mult)
                    nc.vector.tensor_tensor(rq[:, :, 32:64], ra[:], rb[:], OP.add)
                    # transpose 8 heads; even heads batch into one PSUM tile
                    # (-> partitions 0:64), odd heads into another (-> 64:128)
                    dst = qT_sb if which == 0 else kT_sb
                    ptA = psT.tile([64, 4, 128], BF16, tag="pt", name=f"ptA{which}_{t}")
                    ptB = psT.tile([64, 4, 128], BF16, tag="pt", name=f"ptB{which}_{t}")
                    for h in range(HG):
                        pt = ptA if h % 2 == 0 else ptB
                        nc.tensor.transpose(pt[:, h // 2, :], rq[:, h, :], ident[:])
                    nc.scalar.copy(dst[0:64, :, ts(t, 128)], ptA[:])
                    nc.scalar.copy(dst[64:128, :, ts(t, 128)], ptB[:])

                def part_a(t):
                    """ss matmul + x layernorm + FiLM -> h_bf(t)."""
                    embT_strip = work.tile([128, KC, 128], BF16, tag="embT")
                    nc.sync.dma_start(embT_strip[:], embT_r[:, :, ts(t, 128)])
                    scale1p = film.tile([128, C], BF16, tag="scale")
                    shift = film.tile([128, C], BF16, tag="shift")
                    for cc in range(4):
                        ps = psA.tile([128, 512], F32, tag="mm")
                        for ko in range(KC):
                            nc.tensor.matmul(
                                ps[:], lhsT=embT_strip[:, ko, :],
                                rhs=emb_w_sb[:, ko, ts(cc, 512)],
                                start=(ko == 0), stop=(ko == KC - 1))
                        if zero_bias:
                            if cc < 2:
                                # scale + 1
                                nc.scalar.activation(
                                    scale1p[:, ts(cc, 512)], ps[:],
                                    ACTF.Identity, bias=one_col[:], scale=1.0)
                            else:
                                nc.scalar.copy(shift[:, ts(cc - 2, 512)], ps[:])
                        else:
                            if cc < 2:
                                nc.vector.tensor_tensor(
                                    scale1p[:, ts(cc, 512)], ps[:],
                                    b1p_sb[:, ts(cc, 512)], OP.add)
                            else:
                                nc.vector.tensor_tensor(
                                    shift[:, ts(cc - 2, 512)], ps[:],
                                    bsh_sb[:, ts(cc - 2, 512)], OP.add)
                    # layernorm of x over C
                    x_t = work.tile([128, C], F32, tag="x")
                    nc.sync.dma_start(x_t[:], x_r[:, t, :])
                    stats = work.tile([128, 2, 6], F32, tag="stats")
                    for sg in range(2):
                        nc.vector.bn_stats(stats[:, sg, :], x_t[:, ts(sg, 512)])
                    mv = work.tile([128, 2], F32, tag="mv")
                    nc.vector.bn_aggr(mv[:], stats[:])
                    std = work.tile([128, 1], F32, tag="std")
                    nc.scalar.activation(std[:], mv[:, 1:2], ACTF.Sqrt,
                                         bias=eps_sb[:], scale=1.0)
                    nc.vector.reciprocal(std[:], std[:])
                    # LN (f32 -> bf16), then FiLM fully in bf16 (DVE 2x mode)
                    ln_bf = work.tile([128, C], BF16, tag="lnx")
                    nc.vector.tensor_scalar(
                        out=ln_bf[:], in0=x_t[:], scalar1=mv[:, 0:1], scalar2=std[:],
                        op0=OP.subtract, op1=OP.mult)
                    h_bf = work.tile([128, C], BF16, tag="h")
                    nc.vector.tensor_tensor(ln_bf[:], ln_bf[:], scale1p[:], OP.mult)
                    nc.vector.tensor_tensor(h_bf[:], ln_bf[:], shift[:], OP.add)
                    return h_bf

                def part_b(t, h_bf):
                    """h transpose + QKV + QK-LN/RoPE + v for tile t."""
                    # transpose h for this token tile (batched copies)
                    hT_strip = work.tile([128, KC, 128], BF16, tag="hT")
                    for g4 in range(2):
                        pt = psT.tile([128, 4, 128], BF16, tag="pt",
                                      name=f"pth{g4}_{t}")
                        for j in range(4):
                            nc.tensor.transpose(
                                pt[:, j, :], h_bf[:, ts(g4 * 4 + j, 128)], ident[:])
                        nc.scalar.copy(hT_strip[:, ts(g4, 4), :], pt[:])
                    # QKV projection (q, k, v chunks of 512 cols each)
                    for cc in range(3):
                        ps = psA.tile([128, 512], F32, tag="mm")
                        for kc in range(KC):
                            nc.tensor.matmul(
                                ps[:], lhsT=hT_strip[:, kc, :],
                                rhs=proj_sb[:, kc, ts(cc, 512)],
                                start=(kc == 0), stop=(kc == KC - 1))
                        if cc == 2:
                            nc.scalar.copy(
                                v_sb[:, :, t, 0:64],
                                ps.rearrange("p (h d) -> p h d", h=HG))
                        else:
                            process_qk(ps, cc, t)
                    nc.vector.memset(v_sb[:, :, t, 64:65], 1.0)

                # software pipeline: ss(t+1) on PE overlaps FiLM(t) on DVE
                h_prev = None
                for t in range(NT):
                    h_cur = part_a(t)
                    if h_prev is not None:
                        part_b(t - 1, h_prev)
                    h_prev = h_cur
                part_b(NT - 1, h_prev)

            # ================= stage 2: attention ============================
            with (
                tc.tile_pool(name="attw", bufs=2) as attw,
                tc.tile_pool(name="pexp", bufs=4) as pexp,
                tc.tile_pool(name="psS", bufs=3, space="PSUM") as psS,
                tc.tile_pool(name="psO", bufs=2, space="PSUM") as psO,
            ):
                head_tiles = {}
                rec_d = nc.dram_tensor("rec_scratch", [HG, N], BF16)
                rec_d_ap = rec_d[:, :]

                def emit_norm(h):
                    """Divide head h's o^T rows by its softmax rowsums and DMA
                    to the oTn scratch.  The reciprocal row is broadcast to 64
                    partitions via a DRAM bounce (DMA engines are idle here)."""
                    po = 64 * (h % 2)
                    hs = h // 2
                    oT_u, sums = head_tiles.pop(h)
                    nc.vector.reciprocal(sums[:], sums[:])
                    rec_bf = attw.tile([1, N], BF16, tag="recbf", name=f"recbf{h}")
                    nc.vector.tensor_copy(rec_bf[:], sums[:])
                    nc.sync.dma_start(rec_d_ap[h:h + 1, :], rec_bf[:])
                    recb = attw.tile([64, N], BF16, tag="recb", name=f"recb{h}")
                    src = rec_d_ap[h, :]
                    nc.sync.dma_start(
                        recb[:],
                        bass.AP(tensor=src.tensor, offset=src.offset,
                                ap=[[0, 64], src.ap[0]]))
                    nc.vector.tensor_tensor(
                        oTn_sb[po:po + 64, hs, :], oT_u[:], recb[:], OP.mult)

                for h in range(HG):
                    po = 64 * (h % 2)
                    hs = h // 2
                    oT_u = attw.tile([64, N], BF16, tag="oTu", name=f"oTu{h}")
                    sums = attw.tile([1, N], F32, tag="sums", name=f"sums{h}")
                    head_tiles[h] = (oT_u, sums)
                    # q processed in two passes of 1024 cols, so oT holds only
                    # 2 PSUM banks and psS can triple-buffer (PE runs 2 exp
                    # iterations ahead, hiding the cross-engine sem latency)
                    for qp in range(2):
                        oT_ps = [psO.tile([65, 512], F32, tag="oT",
                                          name=f"oT{h}_{qp}_{j}")
                                 for j in range(2)]

                        def emit_av(kt, pT, oT_ps=oT_ps):
                            for j in range(2):
                                nc.tensor.matmul(
                                    oT_ps[j][:], lhsT=v_sb[:, h, kt, :],
                                    rhs=pT[:, ts(j, 512)],
                                    start=(kt == 0), stop=(kt == NT - 1))

                        # av is emitted one step late so exp latency hides
                        # under the next scores matmuls in PE program order
                        pending = None
                        for kt in range(NT):
                            ps_s = psS.tile([128, 1024], F32, tag="s",
                                            name=f"s{h}_{qp}_{kt}")
                            for j in range(2):
                                qc = qp * 2 + j
                                nc.tensor.matmul(
                                    ps_s[:, ts(j, 512)],
                                    lhsT=kT_sb[po:po + 64, hs, ts(kt, 128)],
                                    rhs=qT_sb[po:po + 64, hs, ts(qc, 512)],
                                    start=True, stop=True)
                            pT = pexp.tile([128, 1024], BF16, tag="pT",
                                           name=f"pT{h}_{qp}_{kt}")
                            nc.scalar.activation(pT[:], ps_s[:], ACTF.Exp,
                                                 scale=0.125)
                            if pending is not None:
                                emit_av(*pending)
                            pending = (kt, pT)
                            if qp == 0 and kt == 1 and h > 0:
                                emit_norm(h - 1)
                        emit_av(*pending)
                        # copy out this q-pass's o^T (bf16) + rowsums (f32)
                        for j in range(2):
                            qc = qp * 2 + j
                            nc.vector.tensor_copy(
                                oT_u[:, ts(qc, 512)], oT_ps[j][0:64, :])
                            nc.vector.tensor_copy(
                                sums[:, ts(qc, 512)], oT_ps[j][64:65, :])
                emit_norm(HG - 1)

            # ================= stage 3: output projection ====================
            with (
                tc.tile_pool(name="yw", bufs=4) as yw,
                tc.tile_pool(name="psY", bufs=4, space="PSUM") as psY,
            ):
                for t in range(NT):
                    for cc in range(2):
                        ps_y = psY.tile([128, 512], F32, tag="y")
                        for kc in range(4):
                            nc.tensor.matmul(
                                ps_y[:], lhsT=oTn_sb[:, kc, ts(t, 128)],
                                rhs=weff_sb[:, kc, ts(cc, 512)],
                                start=(kc == 0), stop=(kc == 3))
                        y_t = yw.tile([128, 512], F32, tag="yt")
                        if (t * 2 + cc) % 2 == 0:
                            nc.scalar.copy(y_t[:], ps_y[:])
                        else:
                            nc.vector.tensor_copy(y_t[:], ps_y[:])
                        nc.sync.dma_start(y_r[:, t, ts(cc, 512)], y_t[:])

    nc.compile()
    return nc


_cached = {}


def _get_program(zero_bias):
    if zero_bias not in _cached:
        _cached[zero_bias] = build_program(zero_bias)
    return _cached[zero_bias]


def prepare_in_maps(x, emb, emb_w, emb_b, proj_w, out_w):
    x = np.asarray(x, np.float32)
    emb = np.asarray(emb, np.float32)
    emb_w = np.asarray(emb_w, np.float32)
    emb_b = np.asarray(emb_b, np.float32)
    proj_w = np.asarray(proj_w, np.float32)
    out_w = np.asarray(out_w, np.float32)
    bf = ml_dtypes.bfloat16

    zero_bias = bool(np.all(emb_b == 0.0))
    emb_w_bf = emb_w.astype(bf)
    projs = []
    for g in range(2):
        cols = np.concatenate(
            [proj_w[:, q * C + g * 512: q * C + g * 512 + 512] for q in range(3)],
            axis=1)
        projs.append(np.ascontiguousarray(cols).astype(bf))
    w_eff = np.eye(C, dtype=np.float32) + out_w
    weffs = [np.ascontiguousarray(w_eff[g * 512:(g + 1) * 512, :]).astype(bf)
             for g in range(2)]
    half = D // 2
    inv_freq = (1.0 / (10000.0 ** (np.arange(half, dtype=np.float32) / half)))
    ang = np.arange(N, dtype=np.float32)[:, None] * inv_freq[None, :]
    cos_t = np.cos(ang).astype(bf)
    sin_t = np.sin(ang).astype(bf)
    b1p = (1.0 + emb_b[:C]).astype(np.float32)
    bsh = np.ascontiguousarray(emb_b[C:]).astype(np.float32)

    in_maps = []
    for core in range(8):
        b, g = divmod(core, 2)
        m = {
            "x": np.ascontiguousarray(x[b]),
            "embT": np.ascontiguousarray(emb[b].T).astype(bf),
            "emb_w": emb_w_bf,
            "proj_w": projs[g],
            "w_eff": weffs[g],
            "cos_t": cos_t,
            "sin_t": sin_t,
        }
        if not zero_bias:
            m["bias1p"] = b1p
            m["bias_sh"] = bsh
        in_maps.append(m)
    return in_maps, zero_bias


def kernel(x, emb, emb_w, emb_b, proj_w, out_w, _trace=False):
    in_maps, zero_bias = prepare_in_maps(x, emb, emb_w, emb_b, proj_w, out_w)
    nc = _get_program(zero_bias)
    res = run_bass_kernel_spmd(nc, in_maps, core_ids=list(range(8)),
                               trace=_trace)
    y = np.zeros((B, N, C), np.float32)
    for core in range(8):
        b, g = divmod(core, 2)
        y[b] += res.results[core]["y"]
    if _trace:
        kernel.last_exec_time_ns = res.exec_time_ns
        kernel.last_results = res
    return y
